# revision 18
# baseline (speedup 1.0000x reference)
"""Multi-head causal attention with RoPE on 8 Trainium2 NeuronCores.

Sharding: batch x head-group. Core c owns batch c//4 and heads
[4g, 4g+4) where g = c % 4. QKV projection is column-sliced per core,
attention is fully local per head, and the output projection is
row-parallel: each core writes a full-shape [T, D] partial (bf16) and the
host sums the 4 partials per batch.

On-device layout: q,k live transposed as [head_dim, T] so score tiles are
S^T[k, q], softmax normalization is per-column, and the PV matmul consumes
exp(S^T) directly with v in natural [T, head_dim] layout. All matmuls in
bf16 (1 cycle/row on the PE, 1024-wide moving operands); v carries an
extra ones-column so the PV matmul also produces softmax denominators.
Scores accumulate in [128, 1024] PSUM tiles so exp runs as few, wide
ScalarE activations. DMA dispatch is spread across the Sync (x), ScalarE
(weights/consts) and GpSimd (outputs) queues.
"""
import sys

sys.path.insert(0, "/opt/trn_rl_repo")

import numpy as np

B, T, D, H, HD = 2, 2048, 1024, 16, 64
NCORES = 8
GH = 4  # heads per core
DT = 128  # contraction chunk
NDT = D // DT  # 8
KT = 128  # k-tile (score partition dim)
NKT = T // KT  # 16
QC = 1024  # q-chunk width (score free dim / psum tile width)
NQC = T // QC  # 2

_CACHE = {}


def _build():
    import concourse.bass as bass  # noqa: F401
    from concourse import bacc
    import concourse.mybir as mybir
    from concourse.tile import TileContext

    F32 = mybir.dt.float32
    BF16 = mybir.dt.bfloat16
    AF = mybir.ActivationFunctionType

    nc = bacc.Bacc("TRN2", target_bir_lowering=False)

    XT = nc.dram_tensor("xt", [D, T], BF16, kind="ExternalInput")
    # cols: q01 [0:128] | k01 [128:256] | q23 [256:384] | k23 [384:512] | v [512:768]
    WQKV = nc.dram_tensor("wqkv", [D, 768], BF16, kind="ExternalInput")
    WOUT = nc.dram_tensor("wout", [256, D], BF16, kind="ExternalInput")
    COS = nc.dram_tensor("cos2", [128, T], BF16, kind="ExternalInput")
    SIN = nc.dram_tensor("sin2", [128, T], BF16, kind="ExternalInput")
    P2T = nc.dram_tensor("p2t", [128, 128], BF16, kind="ExternalInput")
    TRIMASK = nc.dram_tensor("trimask", [128, 128], BF16, kind="ExternalInput")
    ONESBC = nc.dram_tensor("onesbc", [1, 64], BF16, kind="ExternalInput")
    OUTP = nc.dram_tensor("outp", [T, D], BF16, kind="ExternalOutput")

    with TileContext(nc) as tc:
        with (
            tc.tile_pool(name="const", bufs=1) as cst,
            tc.tile_pool(name="xt", bufs=1) as xtp,
            tc.tile_pool(name="qk", bufs=1) as qkp,
            tc.tile_pool(name="rt", bufs=2) as rtp,
            tc.tile_pool(name="v", bufs=1) as vp,
            tc.tile_pool(name="pt", bufs=8) as ptp,
            tc.tile_pool(name="sm", bufs=2) as smp,
            tc.tile_pool(name="ot", bufs=1) as otp,
            tc.tile_pool(name="os", bufs=3) as osp,
            tc.tile_pool(name="psS", bufs=2, space="PSUM") as psS,
            tc.tile_pool(name="psPV", bufs=2, space="PSUM") as psPV,
        ):
            # ---- const tiles; DMAs ride the ScalarE (ACT) queue ----
            p2t = cst.tile([128, 128], BF16, tag="p2t")
            nc.scalar.dma_start(p2t[:], P2T[:])  # first: feeds the PE warm-up
            wqkv = []
            for d in range(NDT):
                t_ = cst.tile([DT, 768], BF16, tag=f"wqkv{d}", name=f"wqkv{d}")
                nc.scalar.dma_start(t_[:, 0:256], WQKV[d * DT : (d + 1) * DT, 0:256])
                wqkv.append(t_)
            cos = cst.tile([128, T], BF16, tag="cos")
            sin = cst.tile([128, T], BF16, tag="sin")
            trimask = cst.tile([128, 128], BF16, tag="trimask")
            onesbc = cst.tile([1, 64], BF16, tag="onesbc")
            wout = []
            for c2 in range(2):
                nc.scalar.dma_start(cos[:, c2 * QC : (c2 + 1) * QC],
                                    COS[:, c2 * QC : (c2 + 1) * QC])
                nc.scalar.dma_start(sin[:, c2 * QC : (c2 + 1) * QC],
                                    SIN[:, c2 * QC : (c2 + 1) * QC])
            nc.scalar.dma_start(trimask[:], TRIMASK[:])
            nc.scalar.dma_start(onesbc[:], ONESBC[:])
            for d in range(NDT):
                nc.scalar.dma_start(wqkv[d][:, 256:768],
                                    WQKV[d * DT : (d + 1) * DT, 256:768])
            for g in range(2):
                t_ = cst.tile([128, D], BF16, tag=f"wout{g}", name=f"wout{g}")
                nc.scalar.dma_start(t_[:], WOUT[g * 128 : (g + 1) * 128, :])
                wout.append(t_)

            # ---- x tiles, d-major so QKV can stream; dispatch split across
            # the Sync and GpSimd queues (dispatch rate is the feed limit) ----
            xt = []
            for d in range(NDT):
                t_ = xtp.tile([DT, T], BF16, tag=f"xt{d}", name=f"xt{d}")
                xt.append(t_)
            for d in range(NDT):
                eng = nc.sync if d % 2 == 0 else nc.gpsimd
                for q4 in range(4):
                    eng.dma_start(
                        xt[d][:, q4 * 512 : (q4 + 1) * 512],
                        XT[d * DT : (d + 1) * DT, q4 * 512 : (q4 + 1) * 512],
                    )

            # ---- HAM warm-up: dummy matmuls keep the PE busy while x DMAs
            # land, so the clock gate is at 8/8 when real work starts ----
            with nc.named_scope("warmup"):
                wps = psS.tile([128, QC], F32, tag="big", name="warm")
                for _ in range(56):
                    nc.tensor.matmul(
                        wps[:, 0:128], p2t[:], p2t[:], start=True, stop=True
                    )

            # ---- QKV projection: one pair = 2 heads' worth of q or k ----
            qk = {}

            def project_pair(name, col0):
                dst = qkp.tile([128, T], BF16, tag=name, name=name)
                pss = [
                    psS.tile([128, QC], F32, tag="big", name=f"{name}c{c}")
                    for c in range(NQC)
                ]
                for d in range(NDT):
                    for c in range(NQC):
                        for s in range(2):  # matmul out must fit one PSUM bank
                            nc.tensor.matmul(
                                pss[c][:, s * 512 : (s + 1) * 512],
                                wqkv[d][:, col0 : col0 + 128],
                                xt[d][:, c * QC + s * 512 : c * QC + (s + 1) * 512],
                                start=(d == 0),
                                stop=(d == NDT - 1),
                            )
                for c in range(NQC):
                    nc.vector.tensor_copy(dst[:, c * QC : (c + 1) * QC], pss[c][:])
                qk[name] = dst

            def rope_pair(name):
                raw = qk[name]
                for c in range(NQC):
                    sl = slice(c * QC, (c + 1) * QC)
                    psr = psS.tile([128, QC], F32, tag="big", name=f"r{name}{c}")
                    for s in range(2):
                        nc.tensor.matmul(
                            psr[:, s * 512 : (s + 1) * 512],
                            p2t[:],
                            raw[:, c * QC + s * 512 : c * QC + (s + 1) * 512],
                            start=True,
                            stop=True,
                        )
                    t1 = rtp.tile([128, QC], BF16, tag="t1")
                    nc.vector.tensor_mul(t1[:], psr[:], sin[:, sl])
                    t2 = rtp.tile([128, QC], BF16, tag="t2")
                    nc.vector.tensor_mul(t2[:], raw[:, sl], cos[:, sl])
                    nc.vector.tensor_add(raw[:, sl], t1[:], t2[:])

            with nc.named_scope("qkv01"):
                project_pair("q01", 0)
                project_pair("k01", 128)
            with nc.named_scope("rope01"):
                rope_pair("q01")
                rope_pair("k01")

            # ---- v in natural [tok, vdim] layout, plus ones columns ----
            vt = []
            with nc.named_scope("vproj"):
                for ti in range(NKT):
                    ps = psS.tile([128, QC], F32, tag="big", name=f"v{ti}")
                    for d in range(NDT):
                        nc.tensor.matmul(
                            ps[:, 0:256],
                            xt[d][:, ti * KT : (ti + 1) * KT],
                            wqkv[d][:, 512:768],
                            start=(d == 0),
                            stop=(d == NDT - 1),
                        )
                    v_ = vp.tile([128, 260], BF16, tag=f"v{ti}", name=f"v{ti}")
                    nc.vector.memset(v_[:], 1.0)
                    for h in range(GH):
                        nc.vector.tensor_copy(
                            v_[:, 65 * h : 65 * h + 64], ps[:, 64 * h : 64 * h + 64]
                        )
                    vt.append(v_)

            # ---- attention per head; ot = normalized per-head outputs ----
            ot = [otp.tile([128, T], BF16, tag=f"ot{g}", name=f"ot{g}") for g in range(2)]

            # normalize: row 64 of pso holds sum(exp); fold 1/sum into ot.
            # Deferred so the PE can race ahead into the next chunk's scores
            # before paying the bcast-matmul dependency on the DVE den copy.
            pending_norm = []

            def flush_norm():
                while pending_norm:
                    h, c, pso = pending_norm.pop(0)
                    pair, hr = h // 2, 64 * (h % 2)
                    den = smp.tile([1, QC], BF16, tag="den")
                    nc.vector.tensor_copy(den[:], pso[64:65, :])
                    psb = psS.tile([128, QC], F32, tag="big", name=f"bc{h}{c}")
                    for s in range(2):
                        nc.tensor.matmul(
                            psb[0:64, s * 512 : (s + 1) * 512],
                            onesbc[:],
                            den[:, s * 512 : (s + 1) * 512],
                            start=True,
                            stop=True,
                        )
                    rec = smp.tile([64, QC], F32, tag="rec")
                    nc.vector.reciprocal_approx_fast(rec[:], psb[0:64, :])
                    nc.vector.tensor_mul(
                        ot[pair][hr : hr + 64, c * QC : (c + 1) * QC],
                        pso[0:64, :],
                        rec[:],
                    )

            def attn_pair(h0):
                # interleave heads h0 and h0+1 per k-tile so each engine's
                # stream stays dense (PE never waits a full exp latency)
                pair = h0 // 2
                qT = qk[f"q{'01' if pair == 0 else '23'}"]
                kT = qk[f"k{'01' if pair == 0 else '23'}"]
                for c in (1, 0):  # one q-chunk at a time: 1 psS tile per iter
                    n_i = 8 * c + 8
                    pso = {
                        h: psPV.tile([65, QC], F32, tag="pv", name=f"pso{h}c{c}")
                        for h in (h0, h0 + 1)
                    }
                    fifo = []  # software pipeline: PV trails scores by 1 iter
                    for i in range(n_i):
                        ob = i * KT - c * QC
                        o = max(0, ob)
                        segs = [(o, 512), (512, QC)] if o < 512 else [(o, QC)]
                        pts = {}
                        for h in (h0, h0 + 1):
                            hr = 64 * (h % 2)
                            ps = psS.tile(
                                [128, QC], F32, tag="big", name=f"s{h}_{i}_{c}"
                            )
                            for s0, s1 in segs:
                                nc.tensor.matmul(
                                    ps[:, s0:s1],
                                    kT[hr : hr + 64, i * KT : (i + 1) * KT],
                                    qT[hr : hr + 64, c * QC + s0 : c * QC + s1],
                                    start=True,
                                    stop=True,
                                )
                            if i == 0 and h == h0:
                                flush_norm()
                            pt = ptp.tile(
                                [128, QC], BF16, tag="pt", name=f"pt{h}_{i}_{c}"
                            )
                            nc.scalar.activation(
                                pt[:, o:QC], ps[:, o:QC], AF.Exp, scale=0.125
                            )
                            if ob >= 0:
                                # gpsimd is idle mid-kernel; keeps DVE off the
                                # exp -> PV dependency chain
                                nc.gpsimd.tensor_mul(
                                    pt[:, o : o + 128], pt[:, o : o + 128], trimask[:]
                                )
                            pts[h] = pt
                        fifo.append((i, pts, segs))
                        if len(fifo) > 1:
                            emit_pv(h0, pso, n_i, fifo.pop(0))
                    while fifo:
                        emit_pv(h0, pso, n_i, fifo.pop(0))
                    for h in (h0, h0 + 1):
                        pending_norm.append((h, c, pso[h]))

            def emit_pv(h0, pso, n_i, item):
                i, pts, segs = item
                for h in (h0, h0 + 1):
                    for s0, s1 in segs:
                        nc.tensor.matmul(
                            pso[h][:, s0:s1],
                            vt[i][:, 65 * h : 65 * h + 65],
                            pts[h][:, s0:s1],
                            start=(i == 0),
                            stop=(i == n_i - 1),
                        )

            with nc.named_scope("attn01"):
                attn_pair(0)
            with nc.named_scope("qkv23"):
                project_pair("q23", 256)
                project_pair("k23", 384)
            with nc.named_scope("rope23"):
                rope_pair("q23")
                rope_pair("k23")
            with nc.named_scope("attn23"):
                attn_pair(2)

            # ---- output projection; partial [T, D] written bf16 ----
            with nc.named_scope("oproj"):
                flush_norm()
                for ti in range(NKT):
                    ps = psS.tile([128, QC], F32, tag="big", name=f"o{ti}")
                    for g in range(2):
                        for s in range(2):
                            nc.tensor.matmul(
                                ps[:, s * 512 : (s + 1) * 512],
                                ot[g][:, ti * KT : (ti + 1) * KT],
                                wout[g][:, s * 512 : (s + 1) * 512],
                                start=(g == 0),
                                stop=(g == 1),
                            )
                    osb = osp.tile([128, D], BF16, tag="ost")
                    # ACT is idle after the last exp; halve copy latency by
                    # splitting each across DVE + ACT
                    nc.vector.tensor_copy(osb[:, 0:512], ps[:, 0:512])
                    nc.scalar.copy(osb[:, 512:1024], ps[:, 512:1024])
                    for half in range(2):
                        nc.gpsimd.dma_start(
                            OUTP[ti * KT : (ti + 1) * KT, half * 512 : (half + 1) * 512],
                            osb[:, half * 512 : (half + 1) * 512],
                        )

    nc.compile()
    return nc


def _host_consts(bf16):
    pos = np.arange(T, dtype=np.float64)
    theta = 1.0 / (10000.0 ** (np.arange(0, HD, 2, dtype=np.float64) / HD))
    ang = pos[:, None] * theta[None, :]  # [T, 32]
    cos = np.tile(np.cos(ang), (1, 2)).T  # [64, T]
    sin = np.tile(np.sin(ang), (1, 2)).T
    cos2 = np.vstack([cos, cos]).astype(bf16)  # [128, T] two heads stacked
    sin2 = np.vstack([sin, sin]).astype(bf16)
    # rotate-half as a matmul: rot = P @ q for q in [64, t] column layout
    P = np.zeros((HD, HD), dtype=np.float32)
    for i_ in range(32):
        P[i_, i_ + 32] = -1.0
        P[i_ + 32, i_] = 1.0
    P2 = np.zeros((128, 128), dtype=np.float32)
    P2[0:64, 0:64] = P
    P2[64:128, 64:128] = P
    p2t = np.ascontiguousarray(P2.T).astype(bf16)
    f, p = np.meshgrid(np.arange(128), np.arange(128))
    trimask = (p <= f).astype(bf16)  # [p, f] valid iff p <= f
    onesbc = np.ones((1, 64), dtype=np.float32).astype(bf16)
    return cos2, sin2, p2t, trimask, onesbc


def kernel(x, w_qkv, w_out, b_out):
    import ml_dtypes
    from concourse.bass_utils import run_bass_kernel_spmd

    bf16 = ml_dtypes.bfloat16

    if "nc" not in _CACHE:
        _CACHE["nc"] = _build()
    nc = _CACHE["nc"]

    x = np.asarray(x, dtype=np.float32)
    w_qkv = np.asarray(w_qkv, dtype=np.float32)
    w_out = np.asarray(w_out, dtype=np.float32)
    b_out = np.asarray(b_out, dtype=np.float32)

    cos2, sin2, p2t, trimask, onesbc = _host_consts(bf16)

    wq = w_qkv[:, 0:D]
    wk = w_qkv[:, D : 2 * D]
    wv = w_qkv[:, 2 * D : 3 * D]
    xt_b = [np.ascontiguousarray(x[b].T).astype(bf16) for b in range(B)]

    in_maps = []
    for c in range(NCORES):
        b, g = c // 4, c % 4
        h0 = GH * g  # first head of this core's group
        cs = slice(h0 * HD, h0 * HD + 128)  # heads h0, h0+1
        cs2 = slice(h0 * HD + 128, h0 * HD + 256)  # heads h0+2, h0+3
        vs = slice(h0 * HD, h0 * HD + 256)
        wqkv_c = np.ascontiguousarray(
            np.concatenate([wq[:, cs], wk[:, cs], wq[:, cs2], wk[:, cs2], wv[:, vs]], axis=1)
        ).astype(bf16)  # [D, 768]
        wout_c = np.ascontiguousarray(w_out[vs, :]).astype(bf16)  # [256, D]
        in_maps.append(
            {
                "xt": xt_b[b],
                "wqkv": wqkv_c,
                "wout": wout_c,
                "cos2": cos2,
                "sin2": sin2,
                "p2t": p2t,
                "trimask": trimask,
                "onesbc": onesbc,
            }
        )

    global _last_in_maps
    _last_in_maps = in_maps
    res = run_bass_kernel_spmd(nc, in_maps, list(range(NCORES)))
    out = np.zeros((B, T, D), dtype=np.float64)
    for c in range(NCORES):
        out[c // 4] += np.asarray(res.results[c]["outp"]).astype(np.float64)
    out += b_out.astype(np.float64)
    return out.astype(np.float32)


# revision 29
# speedup vs baseline: 1.0968x; 1.0968x over previous
"""Multi-head causal attention with RoPE on 8 Trainium2 NeuronCores.

Sharding: batch x head-group. Core c owns batch c//4 and heads
[4g, 4g+4) where g = c % 4. QKV projection is column-sliced per core,
attention is fully local per head, and the output projection is
row-parallel: each core writes a full-shape [T, D] partial (bf16) and the
host sums the 4 partials per batch.

On-device layout: q,k live transposed as [head_dim, T] so score tiles are
S^T[k, q], softmax normalization is per-column, and the PV matmul consumes
exp(S^T) directly with v in natural [T, head_dim] layout. All matmuls in
bf16 (1 cycle/row on the PE, 1024-wide moving operands); v carries an
extra ones-column so the PV matmul also produces softmax denominators.
Scores accumulate in [128, 1024] PSUM tiles so exp runs as few, wide
ScalarE activations. DMA dispatch is spread across the Sync (x), ScalarE
(weights/consts) and GpSimd (outputs) queues.
"""
import sys

sys.path.insert(0, "/opt/trn_rl_repo")

import numpy as np

B, T, D, H, HD = 2, 2048, 1024, 16, 64
NCORES = 8
GH = 4  # heads per core
DT = 128  # contraction chunk
NDT = D // DT  # 8
KT = 128  # k-tile (score partition dim)
NKT = T // KT  # 16
QC = 1024  # q-chunk width (score free dim / psum tile width)
NQC = T // QC  # 2

_CACHE = {}


def _build():
    import concourse.bass as bass  # noqa: F401
    from concourse import bacc
    import concourse.mybir as mybir
    from concourse.tile import TileContext

    F32 = mybir.dt.float32
    BF16 = mybir.dt.bfloat16
    AF = mybir.ActivationFunctionType

    nc = bacc.Bacc("TRN2", target_bir_lowering=False)

    XT = nc.dram_tensor("xt", [D, T], BF16, kind="ExternalInput")
    # cols: q01 [0:128] | k01 [128:256] | q23 [256:384] | k23 [384:512] | v [512:768]
    WQKV = nc.dram_tensor("wqkv", [D, 768], BF16, kind="ExternalInput")
    WOUT = nc.dram_tensor("wout", [256, D], BF16, kind="ExternalInput")
    COS = nc.dram_tensor("cos2", [128, T], BF16, kind="ExternalInput")
    SIN = nc.dram_tensor("sin2", [128, T], BF16, kind="ExternalInput")
    P2T = nc.dram_tensor("p2t", [128, 128], BF16, kind="ExternalInput")
    TRIMASK = nc.dram_tensor("trimask", [128, 128], BF16, kind="ExternalInput")
    ONESBC = nc.dram_tensor("onesbc", [1, 64], BF16, kind="ExternalInput")
    OUTP = nc.dram_tensor("outp", [T, D], BF16, kind="ExternalOutput")

    with TileContext(nc) as tc:
        with (
            tc.tile_pool(name="const", bufs=1) as cst,
            tc.tile_pool(name="xt", bufs=1) as xtp,
            tc.tile_pool(name="qk", bufs=1) as qkp,
            tc.tile_pool(name="rt", bufs=2) as rtp,
            tc.tile_pool(name="v", bufs=1) as vp,
            tc.tile_pool(name="pt", bufs=8) as ptp,
            tc.tile_pool(name="sm", bufs=2) as smp,
            tc.tile_pool(name="ot", bufs=1) as otp,
            tc.tile_pool(name="os", bufs=3) as osp,
            tc.tile_pool(name="psS", bufs=2, space="PSUM") as psS,
            tc.tile_pool(name="psPV", bufs=2, space="PSUM") as psPV,
        ):
            # ---- const tiles; DMAs ride the ScalarE (ACT) queue ----
            p2t = cst.tile([128, 128], BF16, tag="p2t")
            nc.scalar.dma_start(p2t[:], P2T[:])  # first: feeds the PE warm-up
            wqkv = []
            for d in range(NDT):
                t_ = cst.tile([DT, 768], BF16, tag=f"wqkv{d}", name=f"wqkv{d}")
                nc.scalar.dma_start(t_[:, 0:256], WQKV[d * DT : (d + 1) * DT, 0:256])
                wqkv.append(t_)
            cos = cst.tile([128, T], BF16, tag="cos")
            sin = cst.tile([128, T], BF16, tag="sin")
            trimask = cst.tile([128, 128], BF16, tag="trimask")
            onesbc = cst.tile([1, 64], BF16, tag="onesbc")
            wout = []
            for c2 in range(2):
                nc.scalar.dma_start(cos[:, c2 * QC : (c2 + 1) * QC],
                                    COS[:, c2 * QC : (c2 + 1) * QC])
                nc.scalar.dma_start(sin[:, c2 * QC : (c2 + 1) * QC],
                                    SIN[:, c2 * QC : (c2 + 1) * QC])
            nc.scalar.dma_start(trimask[:], TRIMASK[:])
            nc.scalar.dma_start(onesbc[:], ONESBC[:])
            for d in range(NDT):
                nc.scalar.dma_start(wqkv[d][:, 256:768],
                                    WQKV[d * DT : (d + 1) * DT, 256:768])
            for g in range(2):
                t_ = cst.tile([128, D], BF16, tag=f"wout{g}", name=f"wout{g}")
                nc.scalar.dma_start(t_[:], WOUT[g * 128 : (g + 1) * 128, :])
                wout.append(t_)

            # ---- x tiles, d-major so QKV can stream; dispatch split across
            # the Sync and GpSimd queues (dispatch rate is the feed limit) ----
            xt = []
            for d in range(NDT):
                t_ = xtp.tile([DT, T], BF16, tag=f"xt{d}", name=f"xt{d}")
                xt.append(t_)
            for d in range(NDT):
                eng = nc.sync if d % 2 == 0 else nc.gpsimd
                for q4 in range(4):
                    eng.dma_start(
                        xt[d][:, q4 * 512 : (q4 + 1) * 512],
                        XT[d * DT : (d + 1) * DT, q4 * 512 : (q4 + 1) * 512],
                    )

            # ---- HAM warm-up: dummy matmuls keep the PE busy while x DMAs
            # land, so the clock gate is at 8/8 when real work starts ----
            with nc.named_scope("warmup"):
                wps = psS.tile([128, QC], F32, tag="big", name="warm")
                for _ in range(56):
                    nc.tensor.matmul(
                        wps[:, 0:128], p2t[:], p2t[:], start=True, stop=True
                    )

            # ---- QKV projection: one pair = 2 heads' worth of q or k ----
            qk = {}

            def project_pair(name, col0):
                dst = qkp.tile([128, T], BF16, tag=name, name=name)
                pss = [
                    psS.tile([128, QC], F32, tag="big", name=f"{name}c{c}")
                    for c in range(NQC)
                ]
                for d in range(NDT):
                    for c in range(NQC):
                        for s in range(2):  # matmul out must fit one PSUM bank
                            nc.tensor.matmul(
                                pss[c][:, s * 512 : (s + 1) * 512],
                                wqkv[d][:, col0 : col0 + 128],
                                xt[d][:, c * QC + s * 512 : c * QC + (s + 1) * 512],
                                start=(d == 0),
                                stop=(d == NDT - 1),
                            )
                for c in range(NQC):
                    # split each PSUM->SBUF copy across DVE + ACT (idle here)
                    nc.vector.tensor_copy(
                        dst[:, c * QC : c * QC + 512], pss[c][:, 0:512]
                    )
                    nc.scalar.copy(
                        dst[:, c * QC + 512 : (c + 1) * QC], pss[c][:, 512:QC]
                    )
                qk[name] = dst

            def rope_pair(name):
                raw = qk[name]
                for c in range(NQC):
                    sl = slice(c * QC, (c + 1) * QC)
                    psr = psS.tile([128, QC], F32, tag="big", name=f"r{name}{c}")
                    for s in range(2):
                        nc.tensor.matmul(
                            psr[:, s * 512 : (s + 1) * 512],
                            p2t[:],
                            raw[:, c * QC + s * 512 : c * QC + (s + 1) * 512],
                            start=True,
                            stop=True,
                        )
                    t1 = rtp.tile([128, QC], BF16, tag="t1")
                    nc.vector.tensor_mul(t1[:], psr[:], sin[:, sl])
                    t2 = rtp.tile([128, QC], BF16, tag="t2")
                    nc.vector.tensor_mul(t2[:], raw[:, sl], cos[:, sl])
                    nc.vector.tensor_add(raw[:, sl], t1[:], t2[:])

            with nc.named_scope("qkv01"):
                project_pair("q01", 0)
                project_pair("k01", 128)
            with nc.named_scope("rope01"):
                rope_pair("q01")
                rope_pair("k01")

            # ---- v in natural [tok, vdim] layout, plus ones columns ----
            vt = []
            with nc.named_scope("vproj"):
                for ti in range(NKT):
                    ps = psS.tile([128, QC], F32, tag="big", name=f"v{ti}")
                    for d in range(NDT):
                        nc.tensor.matmul(
                            ps[:, 0:256],
                            xt[d][:, ti * KT : (ti + 1) * KT],
                            wqkv[d][:, 512:768],
                            start=(d == 0),
                            stop=(d == NDT - 1),
                        )
                    v_ = vp.tile([128, 260], BF16, tag=f"v{ti}", name=f"v{ti}")
                    nc.vector.memset(v_[:], 1.0)
                    for h in range(GH):
                        if h % 2 == 0:
                            nc.vector.tensor_copy(
                                v_[:, 65 * h : 65 * h + 64], ps[:, 64 * h : 64 * h + 64]
                            )
                        else:
                            nc.scalar.copy(
                                v_[:, 65 * h : 65 * h + 64], ps[:, 64 * h : 64 * h + 64]
                            )
                    vt.append(v_)

            # ---- attention per head; ot = normalized per-head outputs ----
            ot = [otp.tile([128, T], BF16, tag=f"ot{g}", name=f"ot{g}") for g in range(2)]

            # normalize: row 64 of pso holds sum(exp); fold 1/sum into ot.
            # Deferred so the PE can race ahead into the next chunk's scores
            # before paying the bcast-matmul dependency on the DVE den copy.
            pending_norm = []

            def flush_norm():
                while pending_norm:
                    h, c, pso = pending_norm.pop(0)
                    pair, hr = h // 2, 64 * (h % 2)
                    den = smp.tile([1, QC], BF16, tag="den")
                    nc.vector.tensor_copy(den[:], pso[64:65, :])
                    psb = psS.tile([128, QC], F32, tag="big", name=f"bc{h}{c}")
                    for s in range(2):
                        nc.tensor.matmul(
                            psb[0:64, s * 512 : (s + 1) * 512],
                            onesbc[:],
                            den[:, s * 512 : (s + 1) * 512],
                            start=True,
                            stop=True,
                        )
                    rec = smp.tile([64, QC], F32, tag="rec")
                    nc.vector.reciprocal_approx_fast(rec[:], psb[0:64, :])
                    nc.vector.tensor_mul(
                        ot[pair][hr : hr + 64, c * QC : (c + 1) * QC],
                        pso[0:64, :],
                        rec[:],
                    )

            def attn_pair(h0):
                # interleave heads h0 and h0+1 per k-tile so each engine's
                # stream stays dense (PE never waits a full exp latency)
                pair = h0 // 2
                qT = qk[f"q{'01' if pair == 0 else '23'}"]
                kT = qk[f"k{'01' if pair == 0 else '23'}"]
                for c in (1, 0):  # one q-chunk at a time: 1 psS tile per iter
                    n_i = 8 * c + 8
                    pso = {
                        h: psPV.tile([65, QC], F32, tag="pv", name=f"pso{h}c{c}")
                        for h in (h0, h0 + 1)
                    }
                    fifo = []  # software pipeline: PV trails scores by 1 iter
                    for i in range(n_i):
                        ob = i * KT - c * QC
                        o = max(0, ob)
                        segs = [(o, 512), (512, QC)] if o < 512 else [(o, QC)]
                        pts = {}
                        for h in (h0, h0 + 1):
                            hr = 64 * (h % 2)
                            ps = psS.tile(
                                [128, QC], F32, tag="big", name=f"s{h}_{i}_{c}"
                            )
                            for s0, s1 in segs:
                                nc.tensor.matmul(
                                    ps[:, s0:s1],
                                    kT[hr : hr + 64, i * KT : (i + 1) * KT],
                                    qT[hr : hr + 64, c * QC + s0 : c * QC + s1],
                                    start=True,
                                    stop=True,
                                )
                            if i == 0 and h == h0:
                                flush_norm()
                            pt = ptp.tile(
                                [128, QC], BF16, tag="pt", name=f"pt{h}_{i}_{c}"
                            )
                            nc.scalar.activation(
                                pt[:, o:QC], ps[:, o:QC], AF.Exp, scale=0.125
                            )
                            if ob >= 0:
                                nc.vector.tensor_mul(
                                    pt[:, o : o + 128], pt[:, o : o + 128], trimask[:]
                                )
                            pts[h] = pt
                        fifo.append((i, pts, segs))
                        if len(fifo) > 1:
                            emit_pv(h0, pso, n_i, fifo.pop(0))
                    while fifo:
                        emit_pv(h0, pso, n_i, fifo.pop(0))
                    for h in (h0, h0 + 1):
                        pending_norm.append((h, c, pso[h]))

            def emit_pv(h0, pso, n_i, item):
                i, pts, segs = item
                for h in (h0, h0 + 1):
                    for s0, s1 in segs:
                        nc.tensor.matmul(
                            pso[h][:, s0:s1],
                            vt[i][:, 65 * h : 65 * h + 65],
                            pts[h][:, s0:s1],
                            start=(i == 0),
                            stop=(i == n_i - 1),
                        )

            with nc.named_scope("attn01"):
                attn_pair(0)
            with nc.named_scope("qkv23"):
                project_pair("q23", 256)
                project_pair("k23", 384)
            with nc.named_scope("rope23"):
                rope_pair("q23")
                rope_pair("k23")
            with nc.named_scope("attn23"):
                attn_pair(2)

            # ---- output projection; partial [T, D] written bf16 ----
            with nc.named_scope("oproj"):
                flush_norm()
                for ti in range(NKT):
                    ps = psS.tile([128, QC], F32, tag="big", name=f"o{ti}")
                    for g in range(2):
                        for s in range(2):
                            nc.tensor.matmul(
                                ps[:, s * 512 : (s + 1) * 512],
                                ot[g][:, ti * KT : (ti + 1) * KT],
                                wout[g][:, s * 512 : (s + 1) * 512],
                                start=(g == 0),
                                stop=(g == 1),
                            )
                    osb = osp.tile([128, D], BF16, tag="ost")
                    # ACT is idle after the last exp; halve copy latency by
                    # splitting each across DVE + ACT
                    nc.vector.tensor_copy(osb[:, 0:512], ps[:, 0:512])
                    nc.scalar.copy(osb[:, 512:1024], ps[:, 512:1024])
                    for half in range(2):
                        nc.gpsimd.dma_start(
                            OUTP[ti * KT : (ti + 1) * KT, half * 512 : (half + 1) * 512],
                            osb[:, half * 512 : (half + 1) * 512],
                        )

    nc.compile()
    return nc


def _host_consts(bf16):
    pos = np.arange(T, dtype=np.float64)
    theta = 1.0 / (10000.0 ** (np.arange(0, HD, 2, dtype=np.float64) / HD))
    ang = pos[:, None] * theta[None, :]  # [T, 32]
    cos = np.tile(np.cos(ang), (1, 2)).T  # [64, T]
    sin = np.tile(np.sin(ang), (1, 2)).T
    cos2 = np.vstack([cos, cos]).astype(bf16)  # [128, T] two heads stacked
    sin2 = np.vstack([sin, sin]).astype(bf16)
    # rotate-half as a matmul: rot = P @ q for q in [64, t] column layout
    P = np.zeros((HD, HD), dtype=np.float32)
    for i_ in range(32):
        P[i_, i_ + 32] = -1.0
        P[i_ + 32, i_] = 1.0
    P2 = np.zeros((128, 128), dtype=np.float32)
    P2[0:64, 0:64] = P
    P2[64:128, 64:128] = P
    p2t = np.ascontiguousarray(P2.T).astype(bf16)
    f, p = np.meshgrid(np.arange(128), np.arange(128))
    trimask = (p <= f).astype(bf16)  # [p, f] valid iff p <= f
    onesbc = np.ones((1, 64), dtype=np.float32).astype(bf16)
    return cos2, sin2, p2t, trimask, onesbc


def kernel(x, w_qkv, w_out, b_out):
    import ml_dtypes
    from concourse.bass_utils import run_bass_kernel_spmd

    bf16 = ml_dtypes.bfloat16

    if "nc" not in _CACHE:
        _CACHE["nc"] = _build()
    nc = _CACHE["nc"]

    x = np.asarray(x, dtype=np.float32)
    w_qkv = np.asarray(w_qkv, dtype=np.float32)
    w_out = np.asarray(w_out, dtype=np.float32)
    b_out = np.asarray(b_out, dtype=np.float32)

    cos2, sin2, p2t, trimask, onesbc = _host_consts(bf16)

    wq = w_qkv[:, 0:D]
    wk = w_qkv[:, D : 2 * D]
    wv = w_qkv[:, 2 * D : 3 * D]
    xt_b = [np.ascontiguousarray(x[b].T).astype(bf16) for b in range(B)]

    in_maps = []
    for c in range(NCORES):
        b, g = c // 4, c % 4
        h0 = GH * g  # first head of this core's group
        cs = slice(h0 * HD, h0 * HD + 128)  # heads h0, h0+1
        cs2 = slice(h0 * HD + 128, h0 * HD + 256)  # heads h0+2, h0+3
        vs = slice(h0 * HD, h0 * HD + 256)
        wqkv_c = np.ascontiguousarray(
            np.concatenate([wq[:, cs], wk[:, cs], wq[:, cs2], wk[:, cs2], wv[:, vs]], axis=1)
        ).astype(bf16)  # [D, 768]
        wout_c = np.ascontiguousarray(w_out[vs, :]).astype(bf16)  # [256, D]
        in_maps.append(
            {
                "xt": xt_b[b],
                "wqkv": wqkv_c,
                "wout": wout_c,
                "cos2": cos2,
                "sin2": sin2,
                "p2t": p2t,
                "trimask": trimask,
                "onesbc": onesbc,
            }
        )

    global _last_in_maps
    _last_in_maps = in_maps
    res = run_bass_kernel_spmd(nc, in_maps, list(range(NCORES)))
    out = np.zeros((B, T, D), dtype=np.float64)
    for c in range(NCORES):
        out[c // 4] += np.asarray(res.results[c]["outp"]).astype(np.float64)
    out += b_out.astype(np.float64)
    return out.astype(np.float32)
